# revision 18
# baseline (speedup 1.0000x reference)
"""Single-head causal attention (B=8, T=4096, EMB=1024, HEAD=64) on 8 trn2 cores.

Strategy: data-parallel over batch, one batch element per NeuronCore.

The per-core kernel is scalar-exp-bound (softmax exp runs only on the Scalar
engine at 1 col/cycle: causal T^2/2 elements = 67584 cols ~ 56us @1.2GHz), so
the whole kernel is organized to start exp as early as possible (~3.5us) and
keep the Scalar engine 100% fed:

  - x is host-packed as [128, KCH=8, T] and DMA'd per t-tile j (512 cols,
    1MB, ~2.9us each), so KQ^T for tile 0 is ready ~3.2us in.
  - Per tile j: KQ^T[128, 512] (8 k-chunk matmuls into 1 PSUM bank), then
    V[4 subtiles, 64] (32 matmuls into 1 bank), CAST to SBUF bf16.
  - Scores for s-chunks a<=4j+3 vs t-tile j stream into [128,<=1536] PSUM
    units (3 banks x2 buffered); ScalarE exp's each unit (scale=1/8 folded),
    bf16 out into a per-tile P^T buffer (exact causal widths - no wasted exp
    columns). Diagonal 128x128 blocks masked by 0/1 multiply after exp.
  - PV: per t-subtile i, chain of i+1 matmuls (stationary P^T slice
    [128,128], moving V-with-ones [128,65]) accumulates [O|rowsum]; PSUM
    bank shared with the V accumulator (tag rotation orders V(j) and the
    chains of tile j-1 on one bank). out = O * reciprocal(rowsum).
  - Emission is software-pipelined so the PE never blocks the scalar engine:
    next tile's KQ/V and the previous tile's PV chains are issued between
    score units of the current tile.

PSUM budget (8 banks): scores 2x[128,1536] (6) + KQ [128,512] (1) + V/PV
shared (1).
"""

import numpy as np
import ml_dtypes

B, T, EMB, HEAD = 8, 4096, 1024, 64
KCH = EMB // 128          # 8 contraction chunks
NTT = T // 512            # 8 t-tiles of 512
NTS = T // 128            # 32 t-subtiles / s-chunks of 128
UNIT = 1536               # max score-unit width (3 PSUM banks)
BF16 = ml_dtypes.bfloat16

_CACHE = {}


def _tile_slots(j):
    """[(a, t0, w, pt_off, start_flag), ...] score slots for t-tile j.

    Exact causal widths, packed so every slot either begins at a PSUM bank
    boundary (start=True allowed: clearing the bank wipes nothing live) or
    is the 128-wide slot sharing the bank opened by the 384 one
    (start=False: its elements' has_written bits were cleared by that
    start). Order: 4j+1 full 512 slots, then widths 384,128 (one bank),
    then 256 (own bank, tail unused). pt layout mirrors psum: contiguous."""
    slots = []
    off = 0
    for a in range(4 * j + 1):
        slots.append((a, 512 * j, 512, off, True))
        off += 512
    for a, flag in ((4 * j + 1, True), (4 * j + 3, False), (4 * j + 2, True)):
        w = 512 * (j + 1) - 128 * a
        slots.append((a, 128 * a, w, off, flag))
        off += w
    return slots


def _units(slots):
    """Greedy-pack slots into units of <= UNIT psum columns, whole banks."""
    units = []
    cur, banks = [], 0
    for s in slots:
        nb = 1 if s[4] else 0  # the start=False 128 slot shares its bank
        if cur and (banks + nb) > UNIT // 512:
            units.append(cur)
            cur, banks = [], 0
        cur.append(s)
        banks += nb
    if cur:
        units.append(cur)
    return units


def _build_program():
    import concourse.bacc as bacc
    import concourse.tile as tile
    from concourse import mybir

    fp32 = mybir.dt.float32
    bf16 = mybir.dt.bfloat16
    fp8 = mybir.dt.float8e4
    DR = mybir.MatmulPerfMode.DoubleRow
    EXP = mybir.ActivationFunctionType.Exp

    PTW = 512 * NTS  # 16384: per-tile P^T buffer, slot a at column 512*a

    nc = bacc.Bacc("TRN2", target_bir_lowering=False, debug=False)
    xt_ap = nc.dram_tensor("xt", [128, KCH, T], bf16, kind="ExternalInput").ap()
    w_ap = nc.dram_tensor("w", [128, KCH, 192], bf16, kind="ExternalInput").ap()
    mask_ap = nc.dram_tensor("mask", [128, 128], bf16, kind="ExternalInput").ap()
    o_ap = nc.dram_tensor("o", [128, NTS, HEAD], fp32, kind="ExternalOutput").ap()

    with tile.TileContext(nc) as tc:
        with (
            tc.tile_pool(name="consts", bufs=1) as consts,
            tc.tile_pool(name="xp", bufs=3) as xp,
            tc.tile_pool(name="ptp", bufs=3) as ptp,
            tc.tile_pool(name="outs", bufs=4) as outs,
            tc.tile_pool(name="stg", bufs=2) as stg,
            tc.tile_pool(name="ps_s", bufs=2, space="PSUM") as ps_s,
            tc.tile_pool(name="ps_kq", bufs=1, space="PSUM") as ps_kq,
            tc.tile_pool(name="ps_vpv", bufs=1, space="PSUM") as ps_vpv,
        ):
            # ---------- constants ----------
            w_sb = consts.tile([128, KCH, 192], bf16, tag="w")
            nc.sync.dma_start(out=w_sb, in_=w_ap)
            mask_sb = consts.tile([128, 128], bf16, tag="mask")
            nc.sync.dma_start(out=mask_sb, in_=mask_ap)
            # V with ones column: slot aa is [128, 65], col 64 preset to 1.0
            vt_sb = consts.tile([128, NTS, 65], bf16, tag="vt")
            nc.gpsimd.memset(vt_sb, 1.0)
            # fp8 K^T / Q^T split into two 32-row halves for DoubleRow matmul:
            # k8_sb[p, i, s] = K^T[32i+p, s], q8_sb[p, i, t] = Q^T[32i+p, t]
            k8_sb = consts.tile([32, 2, T], fp8, tag="k8")
            q8_sb = consts.tile([32, 2, T], fp8, tag="q8")
            # warm the exp table so ACT_TABLE_LOAD doesn't hit the first unit
            warm = consts.tile([128, 1], bf16, tag="warm")
            nc.gpsimd.memset(warm, 0.0)
            nc.scalar.activation(warm, warm, EXP, scale=1.0)

            xt_t = {}

            def dma_x(j):
                xt_t[j] = xp.tile([128, KCH, 512], bf16, tag="x", name=f"xt{j}")
                nc.sync.dma_start(out=xt_t[j], in_=xt_ap[:, :, j * 512:(j + 1) * 512])

            kq_ps = {}
            v_ps = {}

            def emit_kq_half(j, half):
                """Half of KQ^T tile j (4 k-chunks); cast + Q-copy on half 1."""
                if half == 0:
                    kq_ps[j] = ps_kq.tile([128, 512], fp32, tag="kq", name=f"kq{j}")
                for k in range(4 * half, 4 * half + 4):
                    nc.tensor.matmul(
                        kq_ps[j],
                        w_sb[:, k, 0:128],
                        xt_t[j][:, k, :],
                        start=(k == 0),
                        stop=(k == KCH - 1),
                    )
                if half == 1:
                    jsl = slice(j * 512, (j + 1) * 512)
                    stage = stg.tile([128, 512], fp8, tag="kq8", name=f"kq8_{j}")
                    nc.vector.tensor_copy(stage, kq_ps[j])
                    for n, dst in enumerate(
                        (k8_sb[:, 0, jsl], k8_sb[:, 1, jsl],
                         q8_sb[:, 0, jsl], q8_sb[:, 1, jsl])
                    ):
                        nc.gpsimd.dma_start(
                            out=dst, in_=stage[32 * n:32 * (n + 1), :]
                        )

            def emit_v_half(j, half):
                """Half of V tile j (4 k-chunks); copy-with-ones on half 1."""
                if half == 0:
                    v_ps[j] = ps_vpv.tile([128, 4, 64], fp32, tag="vpv", name=f"v{j}")
                for k in range(4 * half, 4 * half + 4):
                    for q in range(4):
                        nc.tensor.matmul(
                            v_ps[j][:, q, :],
                            xt_t[j][:, k, q * 128:(q + 1) * 128],
                            w_sb[:, k, 128:192],
                            start=(k == 0 and q == 0),
                            stop=(k == KCH - 1),
                            skip_group_check=True,
                        )
                if half == 1:
                    nc.vector.tensor_copy(vt_sb[:, 4 * j:4 * j + 4, 0:64], v_ps[j])

            pt = {}

            def emit_unit(j, unit):
                uw = sum(s[2] for s in unit)
                base = unit[0][3]
                psu = ps_s.tile([128, UNIT], fp32, tag="s")
                for (a, t0, w, off, start) in unit:
                    nc.tensor.matmul(
                        psu[:, off - base:off - base + w],
                        k8_sb[:, :, a * 128:(a + 1) * 128],
                        q8_sb[:, :, t0:t0 + w],
                        start=start,
                        stop=True,
                        perf_mode=DR,
                        skip_group_check=True,
                    )
                nc.scalar.activation(
                    pt[j][:, base:base + uw], psu[:, 0:uw], EXP, scale=0.125
                )

            def emit_masks(j, slots):
                # diagonal 128-block is the first 128 columns of each of the
                # four slots for chunks 4j..4j+3
                for (a, t0, w, off, start) in slots[-4:]:
                    nc.vector.tensor_mul(
                        pt[j][:, off:off + 128], pt[j][:, off:off + 128], mask_sb
                    )

            ogs = {}

            def emit_chain(i, pool):
                jj = i // 4
                smap = {s[0]: s for s in _tile_slots(jj)}
                po = pool.tile(
                    [128, 65], fp32,
                    tag="vpv" if pool is ps_vpv else "kq", name=f"po{i}",
                )
                for aa in range(i + 1):
                    (_, t0, _, off, _) = smap[aa]
                    col = off + 128 * i - t0
                    nc.tensor.matmul(
                        po,
                        pt[jj][:, col:col + 128],
                        vt_sb[:, aa, :],
                        start=(aa == 0),
                        stop=(aa == i),
                    )
                dr = outs.tile([128, 1], fp32, tag="dr")
                nc.vector.reciprocal(dr, po[:, 64:65])
                nc.vector.tensor_scalar_mul(ogs[jj][:, i % 4, :], po[:, 0:64], dr)

            # ---------- pipeline ----------
            for j in range(3):
                dma_x(j)
            for h in range(2):
                emit_kq_half(0, h)
            for h in range(2):
                emit_v_half(0, h)

            for j in range(NTT):
                slots = _tile_slots(j)
                units = _units(slots)
                pt[j] = ptp.tile([128, PTW], bf16, tag="pt", name=f"pt{j}")

                # PE side-work interleaved between score units so the tensor
                # engine never idles (keeps the p-state ramped) and the scalar
                # engine is never blocked behind a stalled PE queue.
                work = []
                if j >= 1:
                    ogs[j - 1] = outs.tile(
                        [128, 4, 64], fp32, tag="og", name=f"og{j - 1}"
                    )
                    for i in range(4 * (j - 1), 4 * j):
                        work.append(lambda i=i: emit_chain(i, ps_vpv))
                    work.append(lambda j=j: nc.sync.dma_start(
                        out=o_ap[:, 4 * (j - 1):4 * j, :], in_=ogs[j - 1]
                    ))
                if j + 3 < NTT:
                    work.append(lambda j=j: dma_x(j + 3))
                if j + 1 < NTT:
                    for h in range(2):
                        work.append(lambda j=j, h=h: emit_kq_half(j + 1, h))
                    for h in range(2):
                        work.append(lambda j=j, h=h: emit_v_half(j + 1, h))

                done = 0
                for n, u in enumerate(units):
                    emit_unit(j, u)
                    hi = (n + 1) * len(work) // len(units)
                    while done < hi:
                        work[done]()
                        done += 1
                emit_masks(j, slots)

            # tail: chains of tile 7, two banks in parallel (vpv + kq pools)
            ogs[NTT - 1] = outs.tile([128, 4, 64], fp32, tag="og", name="og7")
            for n, i in enumerate(range(4 * (NTT - 1), 4 * NTT)):
                emit_chain(i, ps_vpv if n % 2 == 0 else ps_kq)
            nc.sync.dma_start(out=o_ap[:, 4 * (NTT - 1):4 * NTT, :], in_=ogs[NTT - 1])

    nc.compile()
    return nc


def _get_nc():
    if "nc" not in _CACHE:
        _CACHE["nc"] = _build_program()
    return _CACHE["nc"]


def _prep_inputs(x, W):
    """Host-side packing shared by kernel() and test harnesses."""
    x = np.asarray(x, dtype=np.float32)
    W = np.asarray(W, dtype=np.float32)
    assert x.shape == (B, T, EMB) and W.shape == (EMB, 3 * HEAD)
    # [B, 128, KCH, T]: partition p of chunk k holds x[b, :, 128k+p]
    xt = np.ascontiguousarray(
        x.transpose(0, 2, 1).reshape(B, KCH, 128, T).transpose(0, 2, 1, 3)
    ).astype(BF16)
    w_r = np.ascontiguousarray(
        W.reshape(KCH, 128, 3 * HEAD)
    ).transpose(1, 0, 2).astype(BF16)
    w_r = np.ascontiguousarray(w_r)
    mask = np.triu(np.ones((128, 128), np.float32)).astype(BF16)
    return xt, w_r, mask


def kernel(x, W):
    from concourse.bass_utils import run_bass_kernel_spmd

    xt, w_r, mask = _prep_inputs(x, W)
    nc = _get_nc()
    in_maps = [{"xt": xt[b], "w": w_r, "mask": mask} for b in range(B)]
    res = run_bass_kernel_spmd(nc, in_maps, list(range(B)))
    # o[p, i, c] = out[128*i + p, c]
    return np.stack(
        [
            res.results[b]["o"].transpose(1, 0, 2).reshape(T, HEAD)
            for b in range(B)
        ]
    ).astype(np.float32)


# revision 19
# speedup vs baseline: 1.2908x; 1.2908x over previous
"""Single-head causal attention (B=8, T=4096, EMB=1024, HEAD=64) on 8 trn2 cores.

Strategy: data-parallel over batch, one batch element per NeuronCore.

The per-core kernel is scalar-exp-bound (softmax exp runs only on the Scalar
engine at 1 col/cycle: causal T^2/2 elements = 67584 cols ~ 56us @1.2GHz), so
the whole kernel is organized to start exp as early as possible (~3.5us) and
keep the Scalar engine 100% fed:

  - x is host-packed as [128, KCH=8, T] and DMA'd per t-tile j (512 cols,
    1MB, ~2.9us each), so KQ^T for tile 0 is ready ~3.2us in.
  - Per tile j: KQ^T[128, 512] (8 k-chunk matmuls into 1 PSUM bank), then
    V[4 subtiles, 64] (32 matmuls into 1 bank), CAST to SBUF bf16.
  - Scores for s-chunks a<=4j+3 vs t-tile j stream into [128,<=1536] PSUM
    units (3 banks x2 buffered); ScalarE exp's each unit (scale=1/8 folded),
    bf16 out into a per-tile P^T buffer (exact causal widths - no wasted exp
    columns). Diagonal 128x128 blocks masked by 0/1 multiply after exp.
  - PV: per t-subtile i, chain of i+1 matmuls (stationary P^T slice
    [128,128], moving V-with-ones [128,65]) accumulates [O|rowsum]; PSUM
    bank shared with the V accumulator (tag rotation orders V(j) and the
    chains of tile j-1 on one bank). out = O * reciprocal(rowsum).
  - Emission is software-pipelined so the PE never blocks the scalar engine:
    next tile's KQ/V and the previous tile's PV chains are issued between
    score units of the current tile.

PSUM budget (8 banks): scores 2x[128,1536] (6) + KQ [128,512] (1) + V/PV
shared (1).
"""

import numpy as np
import ml_dtypes

B, T, EMB, HEAD = 8, 4096, 1024, 64
KCH = EMB // 128          # 8 contraction chunks
NTT = T // 512            # 8 t-tiles of 512
NTS = T // 128            # 32 t-subtiles / s-chunks of 128
UNIT = 1536               # max score-unit width (3 PSUM banks)
BF16 = ml_dtypes.bfloat16

_CACHE = {}


def _tile_slots(j):
    """[(a, t0, w, pt_off, start_flag), ...] score slots for t-tile j.

    Exact causal widths, packed so every slot either begins at a PSUM bank
    boundary (start=True allowed: clearing the bank wipes nothing live) or
    is the 128-wide slot sharing the bank opened by the 384 one
    (start=False: its elements' has_written bits were cleared by that
    start). Order: 4j+1 full 512 slots, then widths 384,128 (one bank),
    then 256 (own bank, tail unused). pt layout mirrors psum: contiguous."""
    slots = []
    off = 0
    for a in range(4 * j + 1):
        slots.append((a, 512 * j, 512, off, True))
        off += 512
    for a, flag in ((4 * j + 1, True), (4 * j + 3, False), (4 * j + 2, True)):
        w = 512 * (j + 1) - 128 * a
        slots.append((a, 128 * a, w, off, flag))
        off += w
    return slots


def _units(slots):
    """Greedy-pack slots into units of <= UNIT psum columns, whole banks."""
    units = []
    cur, banks = [], 0
    for s in slots:
        nb = 1 if s[4] else 0  # the start=False 128 slot shares its bank
        if cur and (banks + nb) > UNIT // 512:
            units.append(cur)
            cur, banks = [], 0
        cur.append(s)
        banks += nb
    if cur:
        units.append(cur)
    return units


def _build_program():
    import concourse.bacc as bacc
    import concourse.tile as tile
    from concourse import mybir

    fp32 = mybir.dt.float32
    bf16 = mybir.dt.bfloat16
    fp8 = mybir.dt.float8e4
    DR = mybir.MatmulPerfMode.DoubleRow
    EXP = mybir.ActivationFunctionType.Exp

    PTW = 512 * NTS  # 16384: per-tile P^T buffer, slot a at column 512*a

    nc = bacc.Bacc("TRN2", target_bir_lowering=False, debug=False)
    xt_ap = nc.dram_tensor("xt", [128, KCH, T], bf16, kind="ExternalInput").ap()
    w_ap = nc.dram_tensor("w", [128, KCH, 192], bf16, kind="ExternalInput").ap()
    mask_ap = nc.dram_tensor("mask", [128, 128], bf16, kind="ExternalInput").ap()
    o_ap = nc.dram_tensor("o", [128, NTS, HEAD], fp32, kind="ExternalOutput").ap()

    with tile.TileContext(nc) as tc:
        with (
            tc.tile_pool(name="consts", bufs=1) as consts,
            tc.tile_pool(name="xp", bufs=3) as xp,
            tc.tile_pool(name="ptp", bufs=3) as ptp,
            tc.tile_pool(name="outs", bufs=4) as outs,
            tc.tile_pool(name="ps_s", bufs=2, space="PSUM") as ps_s,
            tc.tile_pool(name="ps_kq", bufs=1, space="PSUM") as ps_kq,
            tc.tile_pool(name="ps_vpv", bufs=1, space="PSUM") as ps_vpv,
        ):
            # ---------- constants ----------
            w_sb = consts.tile([128, KCH, 192], bf16, tag="w")
            nc.sync.dma_start(out=w_sb, in_=w_ap)
            mask_sb = consts.tile([128, 128], bf16, tag="mask")
            nc.sync.dma_start(out=mask_sb, in_=mask_ap)
            # V with ones column: slot aa is [128, 65], col 64 preset to 1.0
            vt_sb = consts.tile([128, NTS, 65], bf16, tag="vt")
            nc.gpsimd.memset(vt_sb, 1.0)
            kq_sb = consts.tile([128, T], bf16, tag="kq")
            qk_sb = consts.tile([64, T], bf16, tag="qk")
            # warm the exp table so ACT_TABLE_LOAD doesn't hit the first unit
            warm = consts.tile([128, 1], bf16, tag="warm")
            nc.gpsimd.memset(warm, 0.0)
            nc.scalar.activation(warm, warm, EXP, scale=1.0)

            xt_t = {}

            def dma_x(j):
                xt_t[j] = xp.tile([128, KCH, 512], bf16, tag="x", name=f"xt{j}")
                nc.sync.dma_start(out=xt_t[j], in_=xt_ap[:, :, j * 512:(j + 1) * 512])

            kq_ps = {}
            v_ps = {}

            def emit_kq_half(j, half):
                """Half of KQ^T tile j (4 k-chunks); cast + Q-copy on half 1."""
                if half == 0:
                    kq_ps[j] = ps_kq.tile([128, 512], fp32, tag="kq", name=f"kq{j}")
                for k in range(4 * half, 4 * half + 4):
                    nc.tensor.matmul(
                        kq_ps[j],
                        w_sb[:, k, 0:128],
                        xt_t[j][:, k, :],
                        start=(k == 0),
                        stop=(k == KCH - 1),
                    )
                if half == 1:
                    jsl = slice(j * 512, (j + 1) * 512)
                    nc.vector.tensor_copy(kq_sb[:, jsl], kq_ps[j])
                    nc.sync.dma_start(out=qk_sb[:, jsl], in_=kq_sb[64:128, jsl])

            def emit_v_half(j, half):
                """Half of V tile j (4 k-chunks); copy-with-ones on half 1."""
                if half == 0:
                    v_ps[j] = ps_vpv.tile([128, 4, 64], fp32, tag="vpv", name=f"v{j}")
                for k in range(4 * half, 4 * half + 4):
                    for q in range(4):
                        nc.tensor.matmul(
                            v_ps[j][:, q, :],
                            xt_t[j][:, k, q * 128:(q + 1) * 128],
                            w_sb[:, k, 128:192],
                            start=(k == 0 and q == 0),
                            stop=(k == KCH - 1),
                            skip_group_check=True,
                        )
                if half == 1:
                    nc.vector.tensor_copy(vt_sb[:, 4 * j:4 * j + 4, 0:64], v_ps[j])

            pt = {}

            def emit_unit(j, unit):
                uw = sum(s[2] for s in unit)
                base = unit[0][3]
                psu = ps_s.tile([128, UNIT], fp32, tag="s")
                with tc.high_priority():
                    for (a, t0, w, off, start) in unit:
                        nc.tensor.matmul(
                            psu[:, off - base:off - base + w],
                            kq_sb[0:64, a * 128:(a + 1) * 128],
                            qk_sb[:, t0:t0 + w],
                            start=start,
                            stop=True,
                            skip_group_check=True,
                        )
                nc.scalar.activation(
                    pt[j][:, base:base + uw], psu[:, 0:uw], EXP, scale=0.125
                )

            def emit_masks(j, slots):
                # diagonal 128-block is the first 128 columns of each of the
                # four slots for chunks 4j..4j+3
                for (a, t0, w, off, start) in slots[-4:]:
                    nc.vector.tensor_mul(
                        pt[j][:, off:off + 128], pt[j][:, off:off + 128], mask_sb
                    )

            ogs = {}

            def emit_chain(i, pool):
                jj = i // 4
                smap = {s[0]: s for s in _tile_slots(jj)}
                po = pool.tile(
                    [128, 65], fp32,
                    tag="vpv" if pool is ps_vpv else "kq", name=f"po{i}",
                )
                for aa in range(i + 1):
                    (_, t0, _, off, _) = smap[aa]
                    col = off + 128 * i - t0
                    nc.tensor.matmul(
                        po,
                        pt[jj][:, col:col + 128],
                        vt_sb[:, aa, :],
                        start=(aa == 0),
                        stop=(aa == i),
                    )
                dr = outs.tile([128, 1], fp32, tag="dr")
                nc.vector.reciprocal(dr, po[:, 64:65])
                nc.vector.tensor_scalar_mul(ogs[jj][:, i % 4, :], po[:, 0:64], dr)

            # ---------- pipeline ----------
            for j in range(3):
                dma_x(j)
            for h in range(2):
                emit_kq_half(0, h)
            for h in range(2):
                emit_v_half(0, h)

            for j in range(NTT):
                slots = _tile_slots(j)
                units = _units(slots)
                pt[j] = ptp.tile([128, PTW], bf16, tag="pt", name=f"pt{j}")

                # PE side-work interleaved between score units so the tensor
                # engine never idles (keeps the p-state ramped) and the scalar
                # engine is never blocked behind a stalled PE queue.
                work = []
                if j >= 1:
                    ogs[j - 1] = outs.tile(
                        [128, 4, 64], fp32, tag="og", name=f"og{j - 1}"
                    )
                    for i in range(4 * (j - 1), 4 * j):
                        work.append(lambda i=i: emit_chain(i, ps_vpv))
                    work.append(lambda j=j: nc.sync.dma_start(
                        out=o_ap[:, 4 * (j - 1):4 * j, :], in_=ogs[j - 1]
                    ))
                if j + 3 < NTT:
                    work.append(lambda j=j: dma_x(j + 3))
                if j + 1 < NTT:
                    for h in range(2):
                        work.append(lambda j=j, h=h: emit_kq_half(j + 1, h))
                    for h in range(2):
                        work.append(lambda j=j, h=h: emit_v_half(j + 1, h))

                done = 0
                for n, u in enumerate(units):
                    emit_unit(j, u)
                    hi = (n + 1) * len(work) // len(units)
                    while done < hi:
                        work[done]()
                        done += 1
                emit_masks(j, slots)

            # tail: chains of tile 7, two banks in parallel (vpv + kq pools)
            ogs[NTT - 1] = outs.tile([128, 4, 64], fp32, tag="og", name="og7")
            for n, i in enumerate(range(4 * (NTT - 1), 4 * NTT)):
                emit_chain(i, ps_vpv if n % 2 == 0 else ps_kq)
            nc.sync.dma_start(out=o_ap[:, 4 * (NTT - 1):4 * NTT, :], in_=ogs[NTT - 1])

    nc.compile()
    return nc


def _get_nc():
    if "nc" not in _CACHE:
        _CACHE["nc"] = _build_program()
    return _CACHE["nc"]


def _prep_inputs(x, W):
    """Host-side packing shared by kernel() and test harnesses."""
    x = np.asarray(x, dtype=np.float32)
    W = np.asarray(W, dtype=np.float32)
    assert x.shape == (B, T, EMB) and W.shape == (EMB, 3 * HEAD)
    # [B, 128, KCH, T]: partition p of chunk k holds x[b, :, 128k+p]
    xt = np.ascontiguousarray(
        x.transpose(0, 2, 1).reshape(B, KCH, 128, T).transpose(0, 2, 1, 3)
    ).astype(BF16)
    w_r = np.ascontiguousarray(
        W.reshape(KCH, 128, 3 * HEAD)
    ).transpose(1, 0, 2).astype(BF16)
    w_r = np.ascontiguousarray(w_r)
    mask = np.triu(np.ones((128, 128), np.float32)).astype(BF16)
    return xt, w_r, mask


def kernel(x, W):
    from concourse.bass_utils import run_bass_kernel_spmd

    xt, w_r, mask = _prep_inputs(x, W)
    nc = _get_nc()
    in_maps = [{"xt": xt[b], "w": w_r, "mask": mask} for b in range(B)]
    res = run_bass_kernel_spmd(nc, in_maps, list(range(B)))
    # o[p, i, c] = out[128*i + p, c]
    return np.stack(
        [
            res.results[b]["o"].transpose(1, 0, 2).reshape(T, HEAD)
            for b in range(B)
        ]
    ).astype(np.float32)


# revision 20
# speedup vs baseline: 1.3307x; 1.0309x over previous
"""Single-head causal attention (B=8, T=4096, EMB=1024, HEAD=64) on 8 trn2 cores.

Strategy: data-parallel over batch, one batch element per NeuronCore.

The per-core kernel is scalar-exp-bound (softmax exp runs only on the Scalar
engine at 1 col/cycle: causal T^2/2 elements = 67584 cols ~ 56us @1.2GHz), so
the whole kernel is organized to start exp as early as possible (~3.5us) and
keep the Scalar engine 100% fed:

  - x is host-packed as [128, KCH=8, T] and DMA'd per t-tile j (512 cols,
    1MB, ~2.9us each), so KQ^T for tile 0 is ready ~3.2us in.
  - Per tile j: KQ^T[128, 512] (8 k-chunk matmuls into 1 PSUM bank), then
    V[4 subtiles, 64] (32 matmuls into 1 bank), CAST to SBUF bf16.
  - Scores for s-chunks a<=4j+3 vs t-tile j stream into [128,<=1536] PSUM
    units (3 banks x2 buffered); ScalarE exp's each unit (scale=1/8 folded),
    bf16 out into a per-tile P^T buffer (exact causal widths - no wasted exp
    columns). Diagonal 128x128 blocks masked by 0/1 multiply after exp.
  - PV: per t-subtile i, chain of i+1 matmuls (stationary P^T slice
    [128,128], moving V-with-ones [128,65]) accumulates [O|rowsum]; PSUM
    bank shared with the V accumulator (tag rotation orders V(j) and the
    chains of tile j-1 on one bank). out = O * reciprocal(rowsum).
  - Emission is software-pipelined so the PE never blocks the scalar engine:
    next tile's KQ/V and the previous tile's PV chains are issued between
    score units of the current tile.

PSUM budget (8 banks): scores 2x[128,1536] (6) + KQ [128,512] (1) + V/PV
shared (1).
"""

import numpy as np
import ml_dtypes

B, T, EMB, HEAD = 8, 4096, 1024, 64
KCH = EMB // 128          # 8 contraction chunks
NTT = T // 512            # 8 t-tiles of 512
NTS = T // 128            # 32 t-subtiles / s-chunks of 128
UNIT = 1536               # max score-unit width (3 PSUM banks)
BF16 = ml_dtypes.bfloat16

_CACHE = {}


def _tile_slots(j):
    """[(a, t0, w, pt_off, start_flag), ...] score slots for t-tile j.

    Exact causal widths, packed so every slot either begins at a PSUM bank
    boundary (start=True allowed: clearing the bank wipes nothing live) or
    is the 128-wide slot sharing the bank opened by the 384 one
    (start=False: its elements' has_written bits were cleared by that
    start). Order: 4j+1 full 512 slots, then widths 384,128 (one bank),
    then 256 (own bank, tail unused). pt layout mirrors psum: contiguous."""
    slots = []
    off = 0
    for a in range(4 * j + 1):
        slots.append((a, 512 * j, 512, off, True))
        off += 512
    for a, flag in ((4 * j + 1, True), (4 * j + 3, False), (4 * j + 2, True)):
        w = 512 * (j + 1) - 128 * a
        slots.append((a, 128 * a, w, off, flag))
        off += w
    return slots


def _units(slots):
    """Greedy-pack slots into units of <= UNIT psum columns, whole banks."""
    units = []
    cur, banks = [], 0
    for s in slots:
        nb = 1 if s[4] else 0  # the start=False 128 slot shares its bank
        if cur and (banks + nb) > UNIT // 512:
            units.append(cur)
            cur, banks = [], 0
        cur.append(s)
        banks += nb
    if cur:
        units.append(cur)
    return units


def _build_program():
    import concourse.bacc as bacc
    import concourse.tile as tile
    from concourse import mybir

    fp32 = mybir.dt.float32
    bf16 = mybir.dt.bfloat16
    fp8 = mybir.dt.float8e4
    DR = mybir.MatmulPerfMode.DoubleRow
    EXP = mybir.ActivationFunctionType.Exp

    PTW = 512 * NTS  # 16384: per-tile P^T buffer, slot a at column 512*a

    nc = bacc.Bacc("TRN2", target_bir_lowering=False, debug=False)
    xt_ap = nc.dram_tensor("xt", [128, NTT, KCH, 512], bf16, kind="ExternalInput").ap()
    w_ap = nc.dram_tensor("w", [128, KCH, 192], bf16, kind="ExternalInput").ap()
    mask_ap = nc.dram_tensor("mask", [128, 128], bf16, kind="ExternalInput").ap()
    o_ap = nc.dram_tensor("o", [128, NTS, HEAD], fp32, kind="ExternalOutput").ap()

    with tile.TileContext(nc) as tc:
        with (
            tc.tile_pool(name="consts", bufs=1) as consts,
            tc.tile_pool(name="xp", bufs=3) as xp,
            tc.tile_pool(name="ptp", bufs=4) as ptp,
            tc.tile_pool(name="outs", bufs=4) as outs,
            tc.tile_pool(name="ps_s", bufs=2, space="PSUM") as ps_s,
            tc.tile_pool(name="ps_kq", bufs=1, space="PSUM") as ps_kq,
            tc.tile_pool(name="ps_vpv", bufs=1, space="PSUM") as ps_vpv,
        ):
            # ---------- constants ----------
            w_sb = consts.tile([128, KCH, 192], bf16, tag="w")
            nc.sync.dma_start(out=w_sb, in_=w_ap)
            mask_sb = consts.tile([128, 128], bf16, tag="mask")
            nc.sync.dma_start(out=mask_sb, in_=mask_ap)
            # V with ones column: slot aa is [128, 65], col 64 preset to 1.0
            vt_sb = consts.tile([128, NTS, 65], bf16, tag="vt")
            nc.gpsimd.memset(vt_sb, 1.0)
            kq_sb = consts.tile([128, T], bf16, tag="kq")
            qk_sb = consts.tile([64, T], bf16, tag="qk")
            # warm the exp table so ACT_TABLE_LOAD doesn't hit the first unit
            warm = consts.tile([128, 1], bf16, tag="warm")
            nc.gpsimd.memset(warm, 0.0)
            nc.scalar.activation(warm, warm, EXP, scale=1.0)

            xt_t = {}

            def dma_x(j):
                xt_t[j] = xp.tile([128, KCH, 512], bf16, tag="x", name=f"xt{j}")
                nc.sync.dma_start(out=xt_t[j], in_=xt_ap[:, j, :, :])

            kq_ps = {}
            v_ps = {}

            def emit_kq_half(j, half):
                """Half of KQ^T tile j (4 k-chunks); cast + Q-copy on half 1."""
                if half == 0:
                    kq_ps[j] = ps_kq.tile([128, 512], fp32, tag="kq", name=f"kq{j}")
                for k in range(4 * half, 4 * half + 4):
                    nc.tensor.matmul(
                        kq_ps[j],
                        w_sb[:, k, 0:128],
                        xt_t[j][:, k, :],
                        start=(k == 0),
                        stop=(k == KCH - 1),
                    )
                if half == 1:
                    jsl = slice(j * 512, (j + 1) * 512)
                    nc.vector.tensor_copy(kq_sb[:, jsl], kq_ps[j])
                    nc.sync.dma_start(out=qk_sb[:, jsl], in_=kq_sb[64:128, jsl])

            def emit_v_half(j, half):
                """Half of V tile j (4 k-chunks); copy-with-ones on half 1."""
                if half == 0:
                    v_ps[j] = ps_vpv.tile([128, 4, 64], fp32, tag="vpv", name=f"v{j}")
                for k in range(4 * half, 4 * half + 4):
                    for q in range(4):
                        nc.tensor.matmul(
                            v_ps[j][:, q, :],
                            xt_t[j][:, k, q * 128:(q + 1) * 128],
                            w_sb[:, k, 128:192],
                            start=(k == 0 and q == 0),
                            stop=(k == KCH - 1),
                            skip_group_check=True,
                        )
                if half == 1:
                    nc.vector.tensor_copy(vt_sb[:, 4 * j:4 * j + 4, 0:64], v_ps[j])

            pt = {}

            def emit_unit(j, unit):
                uw = sum(s[2] for s in unit)
                base = unit[0][3]
                psu = ps_s.tile([128, UNIT], fp32, tag="s")
                with tc.high_priority():
                    for (a, t0, w, off, start) in unit:
                        nc.tensor.matmul(
                            psu[:, off - base:off - base + w],
                            kq_sb[0:64, a * 128:(a + 1) * 128],
                            qk_sb[:, t0:t0 + w],
                            start=start,
                            stop=True,
                            skip_group_check=True,
                        )
                nc.scalar.activation(
                    pt[j][:, base:base + uw], psu[:, 0:uw], EXP, scale=0.125
                )

            def emit_masks(j, slots):
                # diagonal 128-block is the first 128 columns of each of the
                # four slots for chunks 4j..4j+3
                for (a, t0, w, off, start) in slots[-4:]:
                    nc.vector.tensor_mul(
                        pt[j][:, off:off + 128], pt[j][:, off:off + 128], mask_sb
                    )

            ogs = {}

            def emit_chain(i, pool):
                jj = i // 4
                smap = {s[0]: s for s in _tile_slots(jj)}
                tag = {id(ps_vpv): "vpv", id(ps_kq): "kq", id(ps_s): "s"}[id(pool)]
                po = pool.tile([128, 65], fp32, tag=tag, name=f"po{i}")
                for aa in range(i + 1):
                    (_, t0, _, off, _) = smap[aa]
                    col = off + 128 * i - t0
                    nc.tensor.matmul(
                        po,
                        pt[jj][:, col:col + 128],
                        vt_sb[:, aa, :],
                        start=(aa == 0),
                        stop=(aa == i),
                    )
                dr = outs.tile([128, 1], fp32, tag="dr")
                nc.vector.reciprocal(dr, po[:, 64:65])
                nc.vector.tensor_scalar_mul(ogs[jj][:, i % 4, :], po[:, 0:64], dr)

            # ---------- pipeline ----------
            for j in range(3):
                dma_x(j)
            for h in range(2):
                emit_kq_half(0, h)
            for h in range(2):
                emit_v_half(0, h)

            for j in range(NTT):
                slots = _tile_slots(j)
                units = _units(slots)
                pt[j] = ptp.tile([128, PTW], bf16, tag="pt", name=f"pt{j}")

                # PE side-work interleaved between score units so the tensor
                # engine never idles (keeps the p-state ramped) and the scalar
                # engine is never blocked behind a stalled PE queue.
                work = []
                if j >= 1:
                    ogs[j - 1] = outs.tile(
                        [128, 4, 64], fp32, tag="og", name=f"og{j - 1}"
                    )
                    for i in range(4 * (j - 1), 4 * j):
                        work.append(lambda i=i: emit_chain(i, ps_vpv))
                    work.append(lambda j=j: nc.sync.dma_start(
                        out=o_ap[:, 4 * (j - 1):4 * j, :], in_=ogs[j - 1]
                    ))
                if j + 3 < NTT:
                    work.append(lambda j=j: dma_x(j + 3))
                if j + 1 < NTT:
                    for h in range(2):
                        work.append(lambda j=j, h=h: emit_kq_half(j + 1, h))
                    for h in range(2):
                        work.append(lambda j=j, h=h: emit_v_half(j + 1, h))

                done = 0
                for n, u in enumerate(units):
                    emit_unit(j, u)
                    hi = (n + 1) * len(work) // len(units)
                    while done < hi:
                        work[done]()
                        done += 1
                emit_masks(j, slots)

            # tail: chains of tile 7, two banks in parallel (vpv + kq pools)
            ogs[NTT - 1] = outs.tile([128, 4, 64], fp32, tag="og", name="og7")
            for n, i in enumerate(range(4 * (NTT - 1), 4 * NTT)):
                emit_chain(i, (ps_vpv, ps_kq, ps_s, ps_s)[n])
            nc.sync.dma_start(out=o_ap[:, 4 * (NTT - 1):4 * NTT, :], in_=ogs[NTT - 1])

    nc.compile()
    return nc


def _get_nc():
    if "nc" not in _CACHE:
        _CACHE["nc"] = _build_program()
    return _CACHE["nc"]


def _prep_inputs(x, W):
    """Host-side packing shared by kernel() and test harnesses."""
    x = np.asarray(x, dtype=np.float32)
    W = np.asarray(W, dtype=np.float32)
    assert x.shape == (B, T, EMB) and W.shape == (EMB, 3 * HEAD)
    # [B, 128, KCH, T]: partition p of chunk k holds x[b, :, 128k+p]
    xt = np.ascontiguousarray(
        x.transpose(0, 2, 1)
        .reshape(B, KCH, 128, NTT, 512)
        .transpose(0, 2, 3, 1, 4)
    ).astype(BF16)
    w_r = np.ascontiguousarray(
        W.reshape(KCH, 128, 3 * HEAD)
    ).transpose(1, 0, 2).astype(BF16)
    w_r = np.ascontiguousarray(w_r)
    mask = np.triu(np.ones((128, 128), np.float32)).astype(BF16)
    return xt, w_r, mask


def kernel(x, W):
    from concourse.bass_utils import run_bass_kernel_spmd

    xt, w_r, mask = _prep_inputs(x, W)
    nc = _get_nc()
    in_maps = [{"xt": xt[b], "w": w_r, "mask": mask} for b in range(B)]
    res = run_bass_kernel_spmd(nc, in_maps, list(range(B)))
    # o[p, i, c] = out[128*i + p, c]
    return np.stack(
        [
            res.results[b]["o"].transpose(1, 0, 2).reshape(T, HEAD)
            for b in range(B)
        ]
    ).astype(np.float32)
